# revision 6
# baseline (speedup 1.0000x reference)
"""Trainium2 Bass kernel for nn_Cls_Loss_42331197670001.

Reference computation (N=128 samples, C=345 classes, A=512 features):
    dataW[n,c,:] = W[c] - W[labels[n]]
    sigma2[n,c]  = Lambda * dataW[n,c] @ Sigma[labels[n]] @ dataW[n,c]^T
    dW_dMean[n,c]= dataW[n,c] . (mean_target-mean_source)[labels[n]]
    aug = y_s + 0.5*sigma2 + Lambda*dW_dMean ;  loss = mean softmax-CE(aug, labels)

Everything depends on the sample n only through its label l, so the heavy
quadratic form is computed once per *unique* label:
    (W_c - W_l) Sigma_l (W_c - W_l)^T = d(l,c) - b(l,c) + 0.5*s(l)
    d(l,c) = [W Sigma_l W^T]_{cc}     <- the only O(C*A*A) term, on device
    b, s, mean-shift, softmax-CE      <- tiny, host numpy in float64

Device kernel (SPMD, floor(U/8) labels per core; the U%8 remainder labels
are done on host BLAS): per label j, with C on the OUTPUT PARTITIONS:
    P  = (W*16) @ (Sigma*64)      fp8 DoubleRow matmuls, PSUM [c, b],
                                  W^T stationary, Sigma streamed at N=512
                                  (streams longer than LDWEIGHTS, so weight
                                  loads hide in the PE reorder window)
    d[c] = sum_b P[c,b]*W[c,b]    one fused custom-DVE AFFINE_MUL_REDUCE
                                  per 128-c tile, straight out of PSUM
The rowdot lands directly in d_sb[:, m, j] -- no column-sum matmuls, no
GPSIMD fold, no PSUM->SBUF->PE round trip; the only per-label engines are
PE (6 matmuls) and DVE (3 fused ops).  fp8 inputs halve DMA and double PE
throughput; power-of-two scales keep e4m3 in its sweet range.  Resulting
error on the final scalar loss is ~1e-5 relative.
"""

import math
import sys

import numpy as np

try:
    import concourse.bass as bass
except ImportError:  # harness runs from a bare directory
    sys.path.insert(0, "/opt/trn_rl_repo")
    import concourse.bass as bass

import ml_dtypes

import concourse.mybir as mybir
import concourse.tile as tile
from concourse import bacc
from concourse.bass import ts
from concourse.bass_utils import run_bass_kernel_spmd

N_CORES = 8
A = 512          # feature dim
C = 345          # class count
C_PAD = 384      # 3 * 128
M_TILES = 3      # c-tiles of 128
A_CHUNKS = A // 128   # 4

W_SCALE = 16.0
S_SCALE = 64.0
OUT_SCALE = W_SCALE * W_SCALE * S_SCALE

FP8 = mybir.dt.float8e4
BF16 = mybir.dt.bfloat16
F32 = mybir.dt.float32
FP8_NP = ml_dtypes.float8_e4m3


def build_nc(u_pc: int) -> bass.Bass:
    """Per core: u_pc labels; dout[p, m, j] = [W Sig_j W^T]_{cc} * OUT_SCALE
    with c = m*128 + p."""
    nc = bacc.Bacc()
    wt8 = nc.dram_tensor("wt8", [128, A_CHUNKS, C_PAD], FP8, kind="ExternalInput")
    w32 = nc.dram_tensor("w32", [128, M_TILES, A], F32, kind="ExternalInput")
    sg = nc.dram_tensor("sg", [u_pc, 128, A_CHUNKS, A], FP8, kind="ExternalInput")
    dout = nc.dram_tensor("dout", [128, M_TILES, u_pc], F32, kind="ExternalOutput")

    half = u_pc // 2

    with tile.TileContext(nc) as tc:
        with (
            tc.tile_pool(name="singles", bufs=1) as singles,
            tc.tile_pool(name="spool", bufs=8) as spool,
            tc.tile_pool(name="scrpool", bufs=2) as scrpool,
            tc.tile_pool(name="psum", bufs=8, space="PSUM") as ppool,
        ):
            # Single queue, in need-order: wt8 -> sg0 -> w32 -> sg1...
            wt8_sb = singles.tile([128, A_CHUNKS, C_PAD], FP8)
            nc.sync.dma_start(out=wt8_sb[:], in_=wt8[:])
            s_first = spool.tile([128, A_CHUNKS, A], FP8, tag="s")
            nc.sync.dma_start(out=s_first[:], in_=sg[0])
            w32_sb = singles.tile([128, M_TILES, A], F32)
            nc.sync.dma_start(out=w32_sb[:], in_=w32[:])

            d_sb = singles.tile([128, M_TILES, u_pc], F32)
            # Throwaway op pre-absorbs the w32 DMA wait outside the hot loop.
            scr0 = singles.tile([128, 1], F32)
            nc.vector.tensor_copy(scr0[:], w32_sb[:, 0, 0:1])

            # HAM pre-warm: keep the PE busy during the initial DMA wait so
            # the 3.4us activity window opens before the real matmul stream.
            warm8 = singles.tile([128, 2, A], FP8)
            nc.vector.memset(warm8[:], 0.25)
            for _ in range(3):
                ps_w = ppool.tile([128, A], F32, tag="ps")
                nc.tensor.matmul(
                    ps_w[:],
                    lhsT=warm8[:, 0:2, ts(0, 128)],
                    rhs=warm8[:],
                    start=True,
                    stop=True,
                    perf_mode=mybir.MatmulPerfMode.DoubleRow,
                )

            for j in range(u_pc):
                if j == 0:
                    s_sb = s_first
                else:
                    s_sb = spool.tile([128, A_CHUNKS, A], FP8, tag="s")
                    nc.sync.dma_start(out=s_sb[:], in_=sg[j])
                scr = scrpool.tile([128, M_TILES, A], BF16, tag="scr")
                for m in range(M_TILES):
                    # P[c, b] = sum_a W^T[a, c] * Sig[a, b]; one PSUM bank
                    # per (label, m) so banks recycle at AMR granularity.
                    ps = ppool.tile([128, A], F32, tag="ps")
                    for k in (0, 2):
                        nc.tensor.matmul(
                            ps[:],
                            lhsT=wt8_sb[:, k : k + 2, ts(m, 128)],
                            rhs=s_sb[:, k : k + 2, :],
                            start=(k == 0),
                            stop=(k == 2),
                            perf_mode=mybir.MatmulPerfMode.DoubleRow,
                        )
                    # Fused mult+reduce straight out of PSUM:
                    # accum = sum_b ps * w32.
                    nc.vector.affine_mul_reduce(
                        out=scr[:, m, :],
                        accum_out=d_sb[:, m, ts(j, 1)],
                        in0=ps[:],
                        in1=w32_sb[:, m, :],
                        scale=1.0,
                        bias=0.0,
                    )
                if j == half and u_pc > 2:
                    nc.sync.dma_start(
                        out=dout[:, :, 0:half], in_=d_sb[:, :, 0:half]
                    )
            nc.sync.dma_start(
                out=dout[:, :, half:], in_=d_sb[:, :, half:]
            )
    nc.compile()
    return nc


def host_pack(fc_weight: np.ndarray, lab_dev: np.ndarray, cov: np.ndarray):
    """Build device inputs (wt8, w32, sg_all)."""
    w_pad = np.zeros((C_PAD, A), np.float32)
    w_pad[:C] = fc_weight
    wt = np.ascontiguousarray(
        w_pad.T.reshape(A_CHUNKS, 128, C_PAD).transpose(1, 0, 2)
    )
    wt8 = (wt * W_SCALE).astype(FP8_NP)
    w32 = np.ascontiguousarray(w_pad.reshape(M_TILES, 128, A).transpose(1, 0, 2))
    sgath = cov[lab_dev]                       # [U_dev, A, A]
    sg_all = (
        np.ascontiguousarray(
            sgath.reshape(-1, A_CHUNKS, 128, A).transpose(0, 2, 1, 3)
        )
        * S_SCALE
    ).astype(FP8_NP)
    return wt8, w32, sg_all


_NC_CACHE: dict[int, bass.Bass] = {}


def _device_dSigma(fc_weight, lab_dev, cov):
    """d_Sigma[l, c] = [W Sigma_l W^T]_{cc} for lab_dev (len = 8*u_pc)."""
    u_pc = len(lab_dev) // N_CORES
    wt8, w32, sg_all = host_pack(fc_weight, lab_dev, cov)

    if u_pc not in _NC_CACHE:
        _NC_CACHE[u_pc] = build_nc(u_pc)
    nc = _NC_CACHE[u_pc]

    in_maps = [
        {
            "wt8": wt8,
            "w32": w32,
            "sg": np.ascontiguousarray(sg_all[i * u_pc : (i + 1) * u_pc]),
        }
        for i in range(N_CORES)
    ]
    res = run_bass_kernel_spmd(nc, in_maps, core_ids=list(range(N_CORES)))
    # dout[p, m, j] -> d[j, m*128+p]
    d = np.concatenate(
        [np.asarray(r["dout"]).transpose(2, 1, 0).reshape(u_pc, C_PAD)
         for r in res.results],
        axis=0,
    )
    return d[:, :C].astype(np.float64) / OUT_SCALE


def kernel(
    fc_weight,
    features_source,
    y_s,
    labels_source,
    Lambda,
    mean_source,
    mean_target,
    covariance_target,
):
    fc_weight = np.asarray(fc_weight, dtype=np.float32)
    y_s = np.asarray(y_s, dtype=np.float32)
    labels = np.asarray(labels_source).astype(np.int64)
    lam = float(np.asarray(Lambda))
    mean_source = np.asarray(mean_source, dtype=np.float32)
    mean_target = np.asarray(mean_target, dtype=np.float32)
    cov = np.asarray(covariance_target, dtype=np.float32)

    n = labels.shape[0]
    uniq, inv = np.unique(labels, return_inverse=True)
    U = len(uniq)
    u_pc = U // N_CORES
    n_dev = u_pc * N_CORES

    d_sigma = np.empty((U, C), np.float64)
    if n_dev:
        d_sigma[:n_dev] = _device_dSigma(fc_weight, uniq[:n_dev], cov)
    if n_dev < U:
        # Remainder labels (U % 8) in host BLAS float32 -- exceeds device
        # precision, negligible host time.
        w32h = fc_weight.astype(np.float32)
        for r in range(n_dev, U):
            p = w32h @ cov[uniq[r]]
            d_sigma[r] = np.einsum("ca,ca->c", p, w32h, dtype=np.float64)

    # Cheap per-unique-label terms in float64 on host.
    w64 = fc_weight.astype(np.float64)
    wl = w64[uniq]                                         # [U, A]
    sg64 = cov[uniq].astype(np.float64)                    # [U, A, A]
    # wv = (Sigma + Sigma^T) @ W_l
    wv = np.einsum("uab,ub->ua", sg64, wl) + np.einsum("uab,ua->ub", sg64, wl)
    b = wv @ w64.T                                         # [U, C]
    s = np.einsum("ua,ua->u", wl, wv)                      # [U] = W_l S W_l^T
    quad = d_sigma - b + 0.5 * s[:, None]                  # [U, C]

    d_mean = (mean_target - mean_source).astype(np.float64)[uniq]  # [U, A]
    g = d_mean @ w64.T                                     # [U, C]
    g_self = np.einsum("ua,ua->u", wl, d_mean)             # [U]

    aug = (
        y_s.astype(np.float64)
        + 0.5 * lam * quad[inv]
        + lam * (g[inv] - g_self[inv][:, None])
    )
    mx = aug.max(axis=1, keepdims=True)
    lse = mx[:, 0] + np.log(np.exp(aug - mx).sum(axis=1))
    nll = lse - aug[np.arange(n), labels]
    return np.array(nll.mean(), dtype=np.float32)


# revision 8
# speedup vs baseline: 1.0038x; 1.0038x over previous
"""Trainium2 Bass kernel for nn_Cls_Loss_42331197670001.

Reference computation (N=128 samples, C=345 classes, A=512 features):
    dataW[n,c,:] = W[c] - W[labels[n]]
    sigma2[n,c]  = Lambda * dataW[n,c] @ Sigma[labels[n]] @ dataW[n,c]^T
    dW_dMean[n,c]= dataW[n,c] . (mean_target-mean_source)[labels[n]]
    aug = y_s + 0.5*sigma2 + Lambda*dW_dMean ;  loss = mean softmax-CE(aug, labels)

Everything depends on the sample n only through its label l, so the heavy
quadratic form is computed once per *unique* label:
    (W_c - W_l) Sigma_l (W_c - W_l)^T = d(l,c) - b(l,c) + 0.5*s(l)
    d(l,c) = [W Sigma_l W^T]_{cc}     <- the only O(C*A*A) term, on device
    b, s, mean-shift, softmax-CE      <- tiny, host numpy in float64

Device kernel (SPMD, floor(U/8) labels per core; the U%8 remainder labels
are done on host BLAS): per label j, with C on the OUTPUT PARTITIONS:
    P  = (W*16) @ (Sigma*64)      fp8 DoubleRow matmuls, PSUM [c, b],
                                  W^T stationary, Sigma streamed at N=512
                                  (streams longer than LDWEIGHTS, so weight
                                  loads hide in the PE reorder window)
    d[c] = sum_b P[c,b]*W[c,b]    one fused custom-DVE AFFINE_MUL_REDUCE
                                  per 128-c tile, straight out of PSUM
The rowdot lands directly in d_sb[:, m, j] -- no column-sum matmuls, no
GPSIMD fold, no PSUM->SBUF->PE round trip; the only per-label engines are
PE (6 matmuls) and DVE (3 fused ops).  fp8 inputs halve DMA and double PE
throughput; power-of-two scales keep e4m3 in its sweet range.  Resulting
error on the final scalar loss is ~1e-5 relative.
"""

import math
import sys

import numpy as np

try:
    import concourse.bass as bass
except ImportError:  # harness runs from a bare directory
    sys.path.insert(0, "/opt/trn_rl_repo")
    import concourse.bass as bass

import ml_dtypes

import concourse.mybir as mybir
import concourse.tile as tile
from concourse import bacc
from concourse.bass import ts
from concourse.bass_utils import run_bass_kernel_spmd

N_CORES = 8
A = 512          # feature dim
C = 345          # class count
C_PAD = 384      # 3 * 128
M_TILES = 3      # c-tiles of 128
A_CHUNKS = A // 128   # 4

W_SCALE = 16.0
S_SCALE = 64.0
OUT_SCALE = W_SCALE * W_SCALE * S_SCALE

FP8 = mybir.dt.float8e4
BF16 = mybir.dt.bfloat16
F32 = mybir.dt.float32
FP8_NP = ml_dtypes.float8_e4m3


def build_nc(u_pc: int) -> bass.Bass:
    """Per core: u_pc labels; dout[p, m, j] = [W Sig_j W^T]_{cc} * OUT_SCALE
    with c = m*128 + p."""
    nc = bacc.Bacc()
    wt8 = nc.dram_tensor("wt8", [128, A_CHUNKS, C_PAD], FP8, kind="ExternalInput")
    w32 = nc.dram_tensor("w32", [128, M_TILES, A], F32, kind="ExternalInput")
    sg = nc.dram_tensor("sg", [u_pc, 128, A_CHUNKS, A], FP8, kind="ExternalInput")
    dout = nc.dram_tensor("dout", [128, M_TILES, u_pc], F32, kind="ExternalOutput")

    half = u_pc // 2

    with tile.TileContext(nc) as tc:
        with (
            tc.tile_pool(name="singles", bufs=1) as singles,
            tc.tile_pool(name="spool", bufs=6) as spool,
            tc.tile_pool(name="scrpool", bufs=2) as scrpool,
            tc.tile_pool(name="psum", bufs=8, space="PSUM") as ppool,
        ):
            # ACT HWDGE queue: the W tensors, ordered by first use --
            # w32's m0 slice gates the first AMR, wt8 gates the first MM.
            w32_sb = singles.tile([128, M_TILES, A], F32)
            nc.scalar.dma_start(out=w32_sb[:, 0:1, :], in_=w32[:, 0:1, :])
            wt8_sb = singles.tile([128, A_CHUNKS, C_PAD], FP8)
            nc.scalar.dma_start(out=wt8_sb[:], in_=wt8[:])
            nc.scalar.dma_start(out=w32_sb[:, 1:3, :], in_=w32[:, 1:3, :])
            # Sync queue: the Sigma stream.
            s_first = spool.tile([128, A_CHUNKS, A], FP8, tag="s")
            nc.sync.dma_start(out=s_first[:], in_=sg[0])

            d_sb = singles.tile([128, M_TILES, u_pc], F32)
            # Throwaway op pre-absorbs the w32-m0 DMA wait off the hot loop.
            scr0 = singles.tile([128, 1], F32)
            nc.vector.tensor_copy(scr0[:], w32_sb[:, 0, 0:1])

            # HAM pre-warm: small DMA-independent matmuls keep the PE busy
            # during the initial DMA wait so the 3.4us activity window opens
            # before the real matmul stream.  WAW reuse of the psum bufs by
            # the real matmuls is same-engine ordered (no semaphores).
            warm8 = singles.tile([128, 2, 128], FP8)
            nc.vector.memset(warm8[:], 0.25)
            for _ in range(7):
                ps_w = ppool.tile([128, A], F32, tag="ps")
                nc.tensor.matmul(
                    ps_w[:, 0:128],
                    lhsT=warm8[:],
                    rhs=warm8[:],
                    start=True,
                    stop=True,
                    perf_mode=mybir.MatmulPerfMode.DoubleRow,
                )

            for j in range(u_pc):
                if j == 0:
                    s_sb = s_first
                else:
                    s_sb = spool.tile([128, A_CHUNKS, A], FP8, tag="s")
                    nc.sync.dma_start(out=s_sb[:], in_=sg[j])
                scr = scrpool.tile([128, M_TILES, A], BF16, tag="scr")
                for m in range(M_TILES):
                    # P[c, b] = sum_a W^T[a, c] * Sig[a, b]; one PSUM bank
                    # per (label, m) so banks recycle at AMR granularity.
                    ps = ppool.tile([128, A], F32, tag="ps")
                    for k in (0, 2):
                        nc.tensor.matmul(
                            ps[:],
                            lhsT=wt8_sb[:, k : k + 2, ts(m, 128)],
                            rhs=s_sb[:, k : k + 2, :],
                            start=(k == 0),
                            stop=(k == 2),
                            perf_mode=mybir.MatmulPerfMode.DoubleRow,
                        )
                    # Fused mult+reduce straight out of PSUM:
                    # accum = sum_b ps * w32.
                    nc.vector.affine_mul_reduce(
                        out=scr[:, m, :],
                        accum_out=d_sb[:, m, ts(j, 1)],
                        in0=ps[:],
                        in1=w32_sb[:, m, :],
                        scale=1.0,
                        bias=0.0,
                    )
                if j == half and u_pc > 2:
                    nc.sync.dma_start(
                        out=dout[:, :, 0:half], in_=d_sb[:, :, 0:half]
                    )
            nc.sync.dma_start(
                out=dout[:, :, half:], in_=d_sb[:, :, half:]
            )
    nc.compile()
    return nc


def host_pack(fc_weight: np.ndarray, lab_dev: np.ndarray, cov: np.ndarray):
    """Build device inputs (wt8, w32, sg_all)."""
    w_pad = np.zeros((C_PAD, A), np.float32)
    w_pad[:C] = fc_weight
    wt = np.ascontiguousarray(
        w_pad.T.reshape(A_CHUNKS, 128, C_PAD).transpose(1, 0, 2)
    )
    wt8 = (wt * W_SCALE).astype(FP8_NP)
    w32 = np.ascontiguousarray(w_pad.reshape(M_TILES, 128, A).transpose(1, 0, 2))
    sgath = cov[lab_dev]                       # [U_dev, A, A]
    sg_all = (
        np.ascontiguousarray(
            sgath.reshape(-1, A_CHUNKS, 128, A).transpose(0, 2, 1, 3)
        )
        * S_SCALE
    ).astype(FP8_NP)
    return wt8, w32, sg_all


_NC_CACHE: dict[int, bass.Bass] = {}


def _device_dSigma(fc_weight, lab_dev, cov):
    """d_Sigma[l, c] = [W Sigma_l W^T]_{cc} for lab_dev (len = 8*u_pc)."""
    u_pc = len(lab_dev) // N_CORES
    wt8, w32, sg_all = host_pack(fc_weight, lab_dev, cov)

    if u_pc not in _NC_CACHE:
        _NC_CACHE[u_pc] = build_nc(u_pc)
    nc = _NC_CACHE[u_pc]

    in_maps = [
        {
            "wt8": wt8,
            "w32": w32,
            "sg": np.ascontiguousarray(sg_all[i * u_pc : (i + 1) * u_pc]),
        }
        for i in range(N_CORES)
    ]
    res = run_bass_kernel_spmd(nc, in_maps, core_ids=list(range(N_CORES)))
    # dout[p, m, j] -> d[j, m*128+p]
    d = np.concatenate(
        [np.asarray(r["dout"]).transpose(2, 1, 0).reshape(u_pc, C_PAD)
         for r in res.results],
        axis=0,
    )
    return d[:, :C].astype(np.float64) / OUT_SCALE


def kernel(
    fc_weight,
    features_source,
    y_s,
    labels_source,
    Lambda,
    mean_source,
    mean_target,
    covariance_target,
):
    fc_weight = np.asarray(fc_weight, dtype=np.float32)
    y_s = np.asarray(y_s, dtype=np.float32)
    labels = np.asarray(labels_source).astype(np.int64)
    lam = float(np.asarray(Lambda))
    mean_source = np.asarray(mean_source, dtype=np.float32)
    mean_target = np.asarray(mean_target, dtype=np.float32)
    cov = np.asarray(covariance_target, dtype=np.float32)

    n = labels.shape[0]
    uniq, inv = np.unique(labels, return_inverse=True)
    U = len(uniq)
    u_pc = U // N_CORES
    n_dev = u_pc * N_CORES

    d_sigma = np.empty((U, C), np.float64)
    if n_dev:
        d_sigma[:n_dev] = _device_dSigma(fc_weight, uniq[:n_dev], cov)
    if n_dev < U:
        # Remainder labels (U % 8) in host BLAS float32 -- exceeds device
        # precision, negligible host time.
        w32h = fc_weight.astype(np.float32)
        for r in range(n_dev, U):
            p = w32h @ cov[uniq[r]]
            d_sigma[r] = np.einsum("ca,ca->c", p, w32h, dtype=np.float64)

    # Cheap per-unique-label terms in float64 on host.
    w64 = fc_weight.astype(np.float64)
    wl = w64[uniq]                                         # [U, A]
    sg64 = cov[uniq].astype(np.float64)                    # [U, A, A]
    # wv = (Sigma + Sigma^T) @ W_l
    wv = np.einsum("uab,ub->ua", sg64, wl) + np.einsum("uab,ua->ub", sg64, wl)
    b = wv @ w64.T                                         # [U, C]
    s = np.einsum("ua,ua->u", wl, wv)                      # [U] = W_l S W_l^T
    quad = d_sigma - b + 0.5 * s[:, None]                  # [U, C]

    d_mean = (mean_target - mean_source).astype(np.float64)[uniq]  # [U, A]
    g = d_mean @ w64.T                                     # [U, C]
    g_self = np.einsum("ua,ua->u", wl, d_mean)             # [U]

    aug = (
        y_s.astype(np.float64)
        + 0.5 * lam * quad[inv]
        + lam * (g[inv] - g_self[inv][:, None])
    )
    mx = aug.max(axis=1, keepdims=True)
    lse = mx[:, 0] + np.log(np.exp(aug - mx).sum(axis=1))
    nll = lse - aug[np.arange(n), labels]
    return np.array(nll.mean(), dtype=np.float32)
